# revision 7
# baseline (speedup 1.0000x reference)
"""TransformerConv MixerBlock (x + TransformerConv(x, edge_index)) on 8 trn2 NeuronCores.

Strategy: kv table rows in degree-sorted order, split at 32768 into lo/hi DRAM
tables so dma_gather's int16 indices reach every row. Destination nodes are
packed into 128-node tiles by (deg_lo, deg_hi) lexsort (rounds of 8 tiles share
slot counts so all cores run one SPMD program). Each core builds the full K/V
table (fp16, 512B rows) from x, then per local tile: a few dma_gather calls
fetch the [128, Klo+Khi] incident kv rows (per-node slots, dummy idx 0 +
mask), alpha/softmax/weighted-sum run as pure vector ops aligned per partition
(no one-hot matmuls), then normalize + skip + residual.
"""
import sys, os, types, math
sys.path.insert(0, '/opt/trn_rl_repo')
import numpy as np

P = 128
D = 128
H = 4
DH = 32
NCORES = 8
B = 32768          # lo/hi table split
SMAX = 8           # max slots (128 idx each) per dma_gather call

_prog_cache = {}


def _ensure_hooks():
    """Best-effort shim of antenv.axon_hooks so trace=True profiling works."""
    try:
        import antenv
        if 'antenv.axon_hooks' not in sys.modules:
            mod = types.ModuleType('antenv.axon_hooks')
            state = {'hook': None}
            mod.set_axon_ntff_profile_hook = lambda h: state.__setitem__('hook', h)
            mod.get_axon_ntff_profile_hook = lambda: state['hook']
            sys.modules['antenv.axon_hooks'] = mod
            antenv.axon_hooks = mod
            from trn_agent_boot.trn_boot import _ntff_profile_via_ctypes
            hook = _ntff_profile_via_ctypes('/opt/axon/libaxon_pjrt.so')
            if hook is not None:
                mod.set_axon_ntff_profile_hook(hook)
    except Exception:
        pass
    try:
        import concourse.bass_utils as bass_utils
        bass_utils.upload_artifacts = lambda tmpdir: tmpdir
    except Exception:
        pass


def _chunks(n):
    """Split n slots into gather chunks of <= SMAX slots."""
    out = []
    j = 0
    while j < n:
        c = min(SMAX, n - j)
        out.append((j, c))
        j += c
    return out


def _wrap16(vals):
    """[128*c] gather list -> wrapped [128, 8*c] int16 (pos i at [i%16, i//16],
    replicated across the 8 groups of 16 partitions)."""
    w = vals.reshape(-1, 16).T.astype(np.int16)      # [16, 8c]
    return np.tile(w, (8, 1))


def _prep(x, edge_index, Wq, bq, Wk, bk, Wv, bv, Wskip, bskip):
    N = x.shape[0]
    E = edge_index.shape[1]
    TPC = (N + NCORES * P - 1) // (NCORES * P)
    NT = NCORES * TPC

    src = np.asarray(edge_index[0], dtype=np.int64)
    dst = np.asarray(edge_index[1], dtype=np.int64)
    deg = np.bincount(dst, minlength=N)

    # --- table rows: degree-sorted; node n -> row trow[n]
    order = np.argsort(-deg, kind='stable')
    trow = np.empty(N, dtype=np.int64)
    trow[order] = np.arange(N)
    x_perm = np.zeros((NT * P, D), dtype=np.float16)
    x_perm[trow] = np.asarray(x, dtype=np.float16)

    # --- per-node lo/hi degrees under the table split
    src_row = trow[src]
    is_lo = src_row < B
    dlo = np.bincount(dst[is_lo], minlength=N)
    dhi = np.bincount(dst[~is_lo], minlength=N)

    # --- dst tiling: lexsort by (dlo desc, dhi desc); position i -> node dkey[i]
    dkey = np.lexsort((-dhi, -dlo))
    dpos = np.empty(N, dtype=np.int64)
    dpos[dkey] = np.arange(N)
    dlo_s = np.zeros(NT * P, dtype=np.int64)
    dhi_s = np.zeros(NT * P, dtype=np.int64)
    dlo_s[:N] = dlo[dkey]
    dhi_s[:N] = dhi[dkey]

    RP = NCORES * P   # 1024 positions per round
    Klo = [int(dlo_s[u * RP:(u + 1) * RP].max()) for u in range(TPC)]
    Khi = [int(dhi_s[u * RP:(u + 1) * RP].max()) for u in range(TPC)]

    # --- per-edge slot assignment ---------------------------------------
    # rank of each edge within its (dst, lo/hi) group
    ord2 = np.lexsort((src_row, np.where(is_lo, 0, 1), dpos[dst]))
    dpos_s = dpos[dst][ord2]
    islo_s = is_lo[ord2]
    srow_s = src_row[ord2]
    grp = dpos_s * 2 + (1 - islo_s)           # lo group first within node
    changes = np.ones(E, dtype=bool)
    changes[1:] = grp[1:] != grp[:-1]
    gstart = np.where(changes)[0]
    gid = np.cumsum(changes) - 1
    rank = np.arange(E) - gstart[gid]

    # idx value: lo -> row, hi -> row - B;   slot j: lo j=rank, hi j=Klo+rank
    u_e = dpos_s // RP
    Klo_e = np.asarray(Klo, dtype=np.int64)[u_e]
    slot = np.where(islo_s, rank, Klo_e + rank)
    ival = np.where(islo_s, srow_s, srow_s - B).astype(np.int16)

    Kmax = max(Klo[u] + Khi[u] for u in range(TPC))
    idx_big = np.zeros((NT * P, Kmax), dtype=np.int16)
    idx_big[dpos_s, slot] = ival
    jr = np.arange(Kmax)[None, :]
    msk_big = np.where(
        (jr < dlo_s[:, None]) |
        ((jr >= np.repeat(Klo, RP)[:, None]) &
         (jr < (np.repeat(Klo, RP)[:, None] + dhi_s[:, None]))),
        np.float16(0.0), np.float16(-10000.0))
    idx_big = idx_big.reshape(NT, P, Kmax)
    msk_big = msk_big.reshape(NT, P, Kmax)

    s = 1.0 / math.sqrt(DH)
    wkT = np.asarray(Wk, dtype=np.float32).T.astype(np.float16).copy()
    wvT = np.asarray(Wv, dtype=np.float32).T.astype(np.float16).copy()
    wqT = (np.asarray(Wq, dtype=np.float32).T * s).astype(np.float16).copy()
    wsT = np.asarray(Wskip, dtype=np.float32).T.astype(np.float16).copy()
    for b in (bq, bk, bv, bskip):
        assert np.abs(np.asarray(b)).max() == 0.0, "nonzero biases not supported"

    # wrapped idx stream width per round
    WU = [8 * (sum(c for _, c in _chunks(Klo[u])) +
               sum(c for _, c in _chunks(Khi[u]))) for u in range(TPC)]
    WTOT = sum(WU)
    KTOT = sum(Klo[u] + Khi[u] for u in range(TPC))

    in_maps = []
    for c in range(NCORES):
        # global tile of (core c, local u) covers sorted positions
        # [u*RP + c*P, u*RP + (c+1)*P)
        tidx = [u * NCORES + 0 for u in range(TPC)]  # placeholder
        idx_w = np.empty((P, WTOT), dtype=np.int16)
        msk_c = np.empty((P, KTOT), dtype=np.float16)
        woff = 0
        koff = 0
        xl_rows = np.empty((TPC * P, D), dtype=np.float16)
        for u in range(TPC):
            p0 = u * RP + c * P
            tile_idx = idx_big.reshape(NT * P, Kmax)[p0:p0 + P]
            tile_msk = msk_big.reshape(NT * P, Kmax)[p0:p0 + P]
            for (j0, csz) in _chunks(Klo[u]):
                vals = tile_idx[:, j0:j0 + csz].T.reshape(-1)  # slot-major
                idx_w[:, woff:woff + 8 * csz] = _wrap16(vals)
                woff += 8 * csz
            for (j0, csz) in _chunks(Khi[u]):
                vals = tile_idx[:, Klo[u] + j0:Klo[u] + j0 + csz].T.reshape(-1)
                idx_w[:, woff:woff + 8 * csz] = _wrap16(vals)
                woff += 8 * csz
            kt = Klo[u] + Khi[u]
            msk_c[:, koff:koff + kt] = tile_msk[:, :kt]
            koff += kt
            rows = dkey[p0:p0 + P] if p0 + P <= N else None
            if rows is None:
                seg = dkey[p0:min(p0 + P, N)]
                blk = np.zeros((P, D), dtype=np.float16)
                blk[:len(seg)] = np.asarray(x, dtype=np.float16)[seg]
                xl_rows[u * P:(u + 1) * P] = blk
            else:
                xl_rows[u * P:(u + 1) * P] = np.asarray(x, dtype=np.float16)[rows]
        assert woff == WTOT and koff == KTOT
        in_maps.append({
            "x_perm": x_perm,
            "x_loc": xl_rows,
            "wkT": wkT, "wvT": wvT, "wqT": wqT, "wsT": wsT,
            "idx_w": idx_w, "msk": msk_c,
        })
    return dict(N=N, E=E, TPC=TPC, NT=NT, Klo=tuple(Klo), Khi=tuple(Khi),
                dkey=dkey, WTOT=WTOT, KTOT=KTOT, in_maps=in_maps)


def _build(TPC, NT, Klo, Khi, WTOT, KTOT):
    import concourse.bass as bass
    import concourse.bacc as bacc
    import concourse.mybir as mybir
    import concourse.tile as tile
    from concourse import library_config

    f16 = mybir.dt.float16
    f32 = mybir.dt.float32
    i16 = mybir.dt.int16
    MUL = mybir.AluOpType.mult
    ADD = mybir.AluOpType.add
    EXP = mybir.ActivationFunctionType.Exp
    COPY = mybir.ActivationFunctionType.Copy
    AXLX = mybir.AxisListType.X

    Kmax = max(Klo[u] + Khi[u] for u in range(TPC))
    NHI = NT * P - B

    nc = bacc.Bacc("TRN2", target_bir_lowering=False, debug=False,
                   num_swdge_queues=4, dynamic_dma_scratch_size=32768)
    x_perm = nc.dram_tensor("x_perm", [NT * P, D], f16, kind="ExternalInput")
    x_loc = nc.dram_tensor("x_loc", [TPC * P, D], f16, kind="ExternalInput")
    wkT = nc.dram_tensor("wkT", [D, D], f16, kind="ExternalInput")
    wvT = nc.dram_tensor("wvT", [D, D], f16, kind="ExternalInput")
    wqT = nc.dram_tensor("wqT", [D, D], f16, kind="ExternalInput")
    wsT = nc.dram_tensor("wsT", [D, D], f16, kind="ExternalInput")
    idx_w = nc.dram_tensor("idx_w", [P, WTOT], i16, kind="ExternalInput")
    msk_d = nc.dram_tensor("msk", [P, KTOT], f16, kind="ExternalInput")
    out_t = nc.dram_tensor("out", [TPC * P, D], f32, kind="ExternalOutput")

    kv_lo = nc.dram_tensor("kv_lo", [B, 256], f16)
    kv_hi = nc.dram_tensor("kv_hi", [NHI, 256], f16)

    NB = 4
    assert NT % NB == 0 and B % (NB * P) == 0

    with tile.TileContext(nc) as tc:
        with (
            tc.tile_pool(name="const", bufs=1) as cp,
            tc.tile_pool(name="sbuf", bufs=4) as sb,
            tc.tile_pool(name="big", bufs=2) as bigp,
            tc.tile_pool(name="psA", bufs=2, space="PSUM") as psA,
        ):
            nc.gpsimd.load_library(library_config.mlp)
            wkv_sb = cp.tile([D, 256], f16, tag="wkv")
            wqs_sb = cp.tile([D, 256], f16, tag="wqs")
            q_loc = cp.tile([P, TPC * D], f16, tag="qloc")
            s_loc = cp.tile([P, TPC * D], f16, tag="sloc")
            nc.sync.dma_start(out=wkv_sb[:, 0:128], in_=wkT[:])
            nc.sync.dma_start(out=wkv_sb[:, 128:256], in_=wvT[:])
            nc.sync.dma_start(out=wqs_sb[:, 0:128], in_=wqT[:])
            nc.sync.dma_start(out=wqs_sb[:, 128:256], in_=wsT[:])

            # ---------------- node phase: kv tables ----------------
            for it in range(NT // NB):
                t0 = it * NB
                xT = sb.tile([P, NB * P], f16, tag="xT")
                nc.sync.dma_start(
                    out=xT[:], in_=x_perm[t0 * P:(t0 + NB) * P, :], transpose=True)
                pkv = psA.tile([P, NB * 256], f32, tag="pbig")
                for b in range(NB):
                    nc.tensor.matmul(pkv[:, b * 256:(b + 1) * 256],
                                     lhsT=xT[:, b * P:(b + 1) * P], rhs=wkv_sb[:],
                                     start=True, stop=True)
                kvt = sb.tile([P, NB * 256], f16, tag="kvt")
                nc.scalar.activation(out=kvt[:], in_=pkv[:], func=COPY)
                r0 = t0 * P
                tgt = (kv_lo[r0:r0 + NB * P, :] if r0 < B
                       else kv_hi[r0 - B:r0 - B + NB * P, :])
                nc.sync.dma_start(
                    out=tgt.rearrange("(b p) c -> p b c", p=P),
                    in_=kvt[:].rearrange("p (b c) -> p b c", c=256))

            # ---------------- local phase: q and skip ----------------
            u = 0
            while u < TPC:
                lb = min(NB, TPC - u)
                xTl = sb.tile([P, NB * P], f16, tag="xT")
                nc.sync.dma_start(
                    out=xTl[:, :lb * P], in_=x_loc[u * P:(u + lb) * P, :],
                    transpose=True)
                pq = psA.tile([P, NB * 256], f32, tag="pbig")
                for b in range(lb):
                    nc.tensor.matmul(pq[:, b * 256:(b + 1) * 256],
                                     lhsT=xTl[:, b * P:(b + 1) * P], rhs=wqs_sb[:],
                                     start=True, stop=True)
                nc.scalar.activation(
                    out=q_loc[:, u * D:(u + lb) * D].rearrange(
                        "p (b c) -> p b c", c=P),
                    in_=pq[:, :lb * 256].rearrange(
                        "p (b c) -> p b c", c=256)[:, :, 0:128], func=COPY)
                xl = sb.tile([P, NB, P], f16, tag="xl")
                nc.sync.dma_start(
                    out=xl[:, :lb, :],
                    in_=x_loc[u * P:(u + lb) * P, :].rearrange(
                        "(b p) c -> p b c", p=P))
                nc.vector.tensor_tensor(
                    out=s_loc[:, u * D:(u + lb) * D].rearrange(
                        "p (b c) -> p b c", c=P),
                    in0=pq[:, :lb * 256].rearrange(
                        "p (b c) -> p b c", c=256)[:, :, 128:256],
                    in1=xl[:, :lb, :], op=ADD)
                u += lb

            # ---------------- edge phase ----------------
            woff = 0
            koff = 0
            gq = 0
            for u in range(TPC):
                KL, KH = Klo[u], Khi[u]
                KT = KL + KH
                if KT == 0:
                    of = sb.tile([P, D], f32, tag="of")
                    nc.scalar.activation(
                        out=of[:], in_=s_loc[:, u * D:(u + 1) * D], func=COPY)
                    nc.sync.dma_start(out=out_t[u * P:(u + 1) * P, :], in_=of[:])
                    continue
                WT = 8 * KT
                idx = sb.tile([P, Kmax * 8], i16, tag="idx")
                nc.sync.dma_start(out=idx[:, :WT],
                                  in_=idx_w[:, woff:woff + WT])
                msk = sb.tile([P, Kmax], f16, tag="msk")
                nc.sync.dma_start(out=msk[:, :KT],
                                  in_=msk_d[:, koff:koff + KT])
                woff += WT
                koff += KT

                kv_g = bigp.tile([P, Kmax, 256], f16, tag="kvg")
                wo = 0
                for (j0, csz) in _chunks(KL):
                    nc.gpsimd.dma_gather(
                        kv_g[:, j0:j0 + csz, :], kv_lo[:, :],
                        idx[:, wo:wo + 8 * csz], csz * P, csz * P, 256,
                        queue_num=gq % 4)
                    gq += 1
                    wo += 8 * csz
                for (j0, csz) in _chunks(KH):
                    nc.gpsimd.dma_gather(
                        kv_g[:, KL + j0:KL + j0 + csz, :], kv_hi[:, :],
                        idx[:, wo:wo + 8 * csz], csz * P, csz * P, 256,
                        queue_num=gq % 4)
                    gq += 1
                    wo += 8 * csz

                qk = bigp.tile([P, Kmax, D], f16, tag="qk")
                nc.vector.tensor_tensor(
                    out=qk[:, :KT, :], in0=kv_g[:, :KT, 0:D],
                    in1=q_loc[:, u * D:(u + 1) * D][:, None, :].to_broadcast(
                        [P, KT, D]), op=MUL)
                alraw = sb.tile([P, Kmax, H], f32, tag="alraw")
                nc.vector.tensor_reduce(
                    out=alraw[:, :KT, :].rearrange("p k h -> p (k h)"),
                    in_=qk[:, :KT, :].rearrange("p k (h e) -> p (k h) e", e=DH),
                    axis=AXLX, op=ADD)
                alpha = sb.tile([P, Kmax, H], f16, tag="alpha")
                nc.vector.tensor_tensor(
                    out=alpha[:, :KT, :], in0=alraw[:, :KT, :],
                    in1=msk[:, :KT, None].to_broadcast([P, KT, H]), op=ADD)
                a_e = sb.tile([P, Kmax, H], f16, tag="a_e")
                nc.scalar.activation(out=a_e[:, :KT, :], in_=alpha[:, :KT, :],
                                     func=EXP)
                av = bigp.tile([P, Kmax, D], f16, tag="av")
                nc.vector.tensor_tensor(
                    out=av[:, :KT, :].rearrange("p k (h e) -> p k h e", e=DH),
                    in0=kv_g[:, :KT, 128:256].rearrange(
                        "p k (h e) -> p k h e", e=DH),
                    in1=a_e[:, :KT, :, None].to_broadcast([P, KT, H, DH]),
                    op=MUL)
                # reduce over the KT slots (strided inner axis)
                ov = sb.tile([P, D], f32, tag="ov")
                nc.vector.tensor_reduce(
                    out=ov[:], in_=av[:, :KT, :].rearrange("p k d -> p d k"),
                    axis=AXLX, op=ADD)
                dn = sb.tile([P, H], f32, tag="dn")
                nc.vector.tensor_reduce(
                    out=dn[:], in_=a_e[:, :KT, :].rearrange("p k h -> p h k"),
                    axis=AXLX, op=ADD)
                rc = sb.tile([P, H], f32, tag="rc")
                nc.vector.tensor_scalar(out=rc[:], in0=dn[:],
                                        scalar1=1e-16, scalar2=None, op0=ADD)
                nc.vector.reciprocal(out=rc[:], in_=rc[:])
                ot = sb.tile([P, D], f32, tag="ot")
                nc.vector.tensor_tensor(
                    out=ot[:].rearrange("p (h e) -> p h e", e=DH),
                    in0=ov[:].rearrange("p (h e) -> p h e", e=DH),
                    in1=rc[:, :, None].to_broadcast([P, H, DH]), op=MUL)
                of = sb.tile([P, D], f32, tag="of")
                nc.vector.tensor_tensor(
                    out=of[:], in0=ot[:], in1=s_loc[:, u * D:(u + 1) * D], op=ADD)
                nc.sync.dma_start(out=out_t[u * P:(u + 1) * P, :], in_=of[:])

    nc.finalize()
    return nc


def _run(inputs, trace=False):
    _ensure_hooks()
    from concourse.bass_utils import run_bass_kernel_spmd

    meta = _prep(**inputs)
    key = (meta['TPC'], meta['NT'], meta['Klo'], meta['Khi'],
           meta['WTOT'], meta['KTOT'])
    if key not in _prog_cache:
        _prog_cache[key] = _build(*key)
    nc = _prog_cache[key]
    res = run_bass_kernel_spmd(nc, meta['in_maps'],
                               core_ids=list(range(NCORES)), trace=trace)
    TPC, N = meta['TPC'], meta['N']
    RP = NCORES * P
    out_sorted = np.empty((TPC * RP, D), dtype=np.float32)
    for c in range(NCORES):
        oc = np.asarray(res.results[c]["out"]).reshape(TPC, P, D)
        for u in range(TPC):
            out_sorted[u * RP + c * P:u * RP + (c + 1) * P] = oc[u]
    out = np.empty((N, D), dtype=np.float32)
    out[meta['dkey']] = out_sorted[:N]
    return out, res


def kernel(**inputs) -> np.ndarray:
    out, _ = _run(inputs, trace=False)
    return out


# revision 14
# speedup vs baseline: 1.3287x; 1.3287x over previous
"""TransformerConv MixerBlock (x + TransformerConv(x, edge_index)) on 8 trn2 NeuronCores.

Strategy: kv table rows in degree-sorted order, split at 32768 into lo/hi DRAM
tables so dma_gather's int16 indices reach every row. Destination nodes are
packed into 128-node tiles by (deg_lo, deg_hi) lexsort (rounds of 8 tiles share
slot counts so all cores run one SPMD program). Each core builds the full K/V
table (fp16, 512B rows) from x, then per local tile: a few dma_gather calls
fetch the [128, Klo+Khi] incident kv rows (per-node slots, dummy idx 0 +
mask), alpha/softmax/weighted-sum run as pure vector ops aligned per partition
(no one-hot matmuls), then normalize + skip + residual.
"""
import sys, os, types, math
sys.path.insert(0, '/opt/trn_rl_repo')
import numpy as np

P = 128
D = 128
H = 4
DH = 32
NCORES = 8
B = 32768          # lo/hi table split
SMAX = 8           # max slots (128 idx each) per dma_gather call

_prog_cache = {}


def _ensure_hooks():
    """Best-effort shim of antenv.axon_hooks so trace=True profiling works."""
    try:
        import antenv
        if 'antenv.axon_hooks' not in sys.modules:
            mod = types.ModuleType('antenv.axon_hooks')
            state = {'hook': None}
            mod.set_axon_ntff_profile_hook = lambda h: state.__setitem__('hook', h)
            mod.get_axon_ntff_profile_hook = lambda: state['hook']
            sys.modules['antenv.axon_hooks'] = mod
            antenv.axon_hooks = mod
            from trn_agent_boot.trn_boot import _ntff_profile_via_ctypes
            hook = _ntff_profile_via_ctypes('/opt/axon/libaxon_pjrt.so')
            if hook is not None:
                mod.set_axon_ntff_profile_hook(hook)
    except Exception:
        pass
    try:
        import concourse.bass_utils as bass_utils
        bass_utils.upload_artifacts = lambda tmpdir: tmpdir
    except Exception:
        pass


def _chunks(n):
    """Split n slots into gather chunks of <= SMAX slots."""
    out = []
    j = 0
    while j < n:
        c = min(SMAX, n - j)
        out.append((j, c))
        j += c
    return out


def _wrap16(vals):
    """[128*c] gather list -> wrapped [128, 8*c] int16 (pos i at [i%16, i//16],
    replicated across the 8 groups of 16 partitions)."""
    w = vals.reshape(-1, 16).T.astype(np.int16)      # [16, 8c]
    return np.tile(w, (8, 1))


def _prep(x, edge_index, Wq, bq, Wk, bk, Wv, bv, Wskip, bskip):
    N = x.shape[0]
    E = edge_index.shape[1]
    TPC = (N + NCORES * P - 1) // (NCORES * P)
    NT = NCORES * TPC

    src = np.asarray(edge_index[0], dtype=np.int64)
    dst = np.asarray(edge_index[1], dtype=np.int64)
    deg = np.bincount(dst, minlength=N)

    # --- table rows: degree-sorted; node n -> row trow[n]
    order = np.argsort(-deg, kind='stable')
    trow = np.empty(N, dtype=np.int64)
    trow[order] = np.arange(N)
    x_perm = np.zeros((NT * P, D), dtype=np.float16)
    x_perm[trow] = np.asarray(x, dtype=np.float16)

    # --- per-node lo/hi degrees under the table split
    src_row = trow[src]
    is_lo = src_row < B
    dlo = np.bincount(dst[is_lo], minlength=N)
    dhi = np.bincount(dst[~is_lo], minlength=N)

    # --- dst tiling: lexsort by (dlo desc, dhi desc); position i -> node dkey[i]
    dkey = np.lexsort((-dhi, -dlo))
    dpos = np.empty(N, dtype=np.int64)
    dpos[dkey] = np.arange(N)
    dlo_s = np.zeros(NT * P, dtype=np.int64)
    dhi_s = np.zeros(NT * P, dtype=np.int64)
    dlo_s[:N] = dlo[dkey]
    dhi_s[:N] = dhi[dkey]

    RP = NCORES * P   # 1024 positions per round
    Klo = [int(dlo_s[u * RP:(u + 1) * RP].max()) for u in range(TPC)]
    Khi = [int(dhi_s[u * RP:(u + 1) * RP].max()) for u in range(TPC)]

    # --- per-edge slot assignment ---------------------------------------
    # rank of each edge within its (dst, lo/hi) group
    ord2 = np.lexsort((src_row, np.where(is_lo, 0, 1), dpos[dst]))
    dpos_s = dpos[dst][ord2]
    islo_s = is_lo[ord2]
    srow_s = src_row[ord2]
    grp = dpos_s * 2 + (1 - islo_s)           # lo group first within node
    changes = np.ones(E, dtype=bool)
    changes[1:] = grp[1:] != grp[:-1]
    gstart = np.where(changes)[0]
    gid = np.cumsum(changes) - 1
    rank = np.arange(E) - gstart[gid]

    # idx value: lo -> row, hi -> row - B;   slot j: lo j=rank, hi j=Klo+rank
    u_e = dpos_s // RP
    Klo_e = np.asarray(Klo, dtype=np.int64)[u_e]
    slot = np.where(islo_s, rank, Klo_e + rank)
    ival = np.where(islo_s, srow_s, srow_s - B).astype(np.int16)

    Kmax = max(Klo[u] + Khi[u] for u in range(TPC))
    idx_big = np.zeros((NT * P, Kmax), dtype=np.int16)
    idx_big[dpos_s, slot] = ival
    jr = np.arange(Kmax)[None, :]
    msk_big = np.where(
        (jr < dlo_s[:, None]) |
        ((jr >= np.repeat(Klo, RP)[:, None]) &
         (jr < (np.repeat(Klo, RP)[:, None] + dhi_s[:, None]))),
        np.float16(0.0), np.float16(-10000.0))
    idx_big = idx_big.reshape(NT, P, Kmax)
    msk_big = msk_big.reshape(NT, P, Kmax)

    s = 1.0 / math.sqrt(DH)
    wkT = np.asarray(Wk, dtype=np.float32).T.astype(np.float16).copy()
    wvT = np.asarray(Wv, dtype=np.float32).T.astype(np.float16).copy()
    wqT = (np.asarray(Wq, dtype=np.float32).T * s).astype(np.float16).copy()
    wsT = np.asarray(Wskip, dtype=np.float32).T.astype(np.float16).copy()
    for b in (bq, bk, bv, bskip):
        assert np.abs(np.asarray(b)).max() == 0.0, "nonzero biases not supported"

    # wrapped idx stream width per round
    WU = [8 * (sum(c for _, c in _chunks(Klo[u])) +
               sum(c for _, c in _chunks(Khi[u]))) for u in range(TPC)]
    WTOT = sum(WU)
    KTOT = sum(Klo[u] + Khi[u] for u in range(TPC))

    in_maps = []
    for c in range(NCORES):
        # global tile of (core c, local u) covers sorted positions
        # [u*RP + c*P, u*RP + (c+1)*P)
        tidx = [u * NCORES + 0 for u in range(TPC)]  # placeholder
        idx_w = np.empty((P, WTOT), dtype=np.int16)
        msk_c = np.empty((P, KTOT), dtype=np.float16)
        woff = 0
        koff = 0
        xl_rows = np.empty((TPC * P, D), dtype=np.float16)
        for u in range(TPC):
            p0 = u * RP + c * P
            tile_idx = idx_big.reshape(NT * P, Kmax)[p0:p0 + P]
            tile_msk = msk_big.reshape(NT * P, Kmax)[p0:p0 + P]
            for (j0, csz) in _chunks(Klo[u]):
                vals = tile_idx[:, j0:j0 + csz].T.reshape(-1)  # slot-major
                idx_w[:, woff:woff + 8 * csz] = _wrap16(vals)
                woff += 8 * csz
            for (j0, csz) in _chunks(Khi[u]):
                vals = tile_idx[:, Klo[u] + j0:Klo[u] + j0 + csz].T.reshape(-1)
                idx_w[:, woff:woff + 8 * csz] = _wrap16(vals)
                woff += 8 * csz
            kt = Klo[u] + Khi[u]
            msk_c[:, koff:koff + kt] = tile_msk[:, :kt]
            koff += kt
            rows = dkey[p0:p0 + P] if p0 + P <= N else None
            if rows is None:
                seg = dkey[p0:min(p0 + P, N)]
                blk = np.zeros((P, D), dtype=np.float16)
                blk[:len(seg)] = np.asarray(x, dtype=np.float16)[seg]
                xl_rows[u * P:(u + 1) * P] = blk
            else:
                xl_rows[u * P:(u + 1) * P] = np.asarray(x, dtype=np.float16)[rows]
        assert woff == WTOT and koff == KTOT
        in_maps.append({
            "x_perm": x_perm,
            "x_loc": xl_rows,
            "wkT": wkT, "wvT": wvT, "wqT": wqT, "wsT": wsT,
            "idx_w": idx_w, "msk": msk_c,
        })
    return dict(N=N, E=E, TPC=TPC, NT=NT, Klo=tuple(Klo), Khi=tuple(Khi),
                dkey=dkey, WTOT=WTOT, KTOT=KTOT, in_maps=in_maps)


def _build(TPC, NT, Klo, Khi, WTOT, KTOT):
    import concourse.bass as bass
    import concourse.bacc as bacc
    import concourse.mybir as mybir
    import concourse.tile as tile
    from concourse import library_config

    f16 = mybir.dt.float16
    f32 = mybir.dt.float32
    i16 = mybir.dt.int16
    MUL = mybir.AluOpType.mult
    ADD = mybir.AluOpType.add
    EXP = mybir.ActivationFunctionType.Exp
    COPY = mybir.ActivationFunctionType.Copy
    AXLX = mybir.AxisListType.X

    Kmax = max(Klo[u] + Khi[u] for u in range(TPC))
    NHI = NT * P - B

    nc = bacc.Bacc("TRN2", target_bir_lowering=False, debug=False,
                   num_swdge_queues=4)
    x_perm = nc.dram_tensor("x_perm", [NT * P, D], f16, kind="ExternalInput")
    x_loc = nc.dram_tensor("x_loc", [TPC * P, D], f16, kind="ExternalInput")
    wkT = nc.dram_tensor("wkT", [D, D], f16, kind="ExternalInput")
    wvT = nc.dram_tensor("wvT", [D, D], f16, kind="ExternalInput")
    wqT = nc.dram_tensor("wqT", [D, D], f16, kind="ExternalInput")
    wsT = nc.dram_tensor("wsT", [D, D], f16, kind="ExternalInput")
    idx_w = nc.dram_tensor("idx_w", [P, WTOT], i16, kind="ExternalInput")
    msk_d = nc.dram_tensor("msk", [P, KTOT], f16, kind="ExternalInput")
    out_t = nc.dram_tensor("out", [TPC * P, D], f32, kind="ExternalOutput")

    kv_lo = nc.dram_tensor("kv_lo", [B, 256], f16)
    kv_hi = nc.dram_tensor("kv_hi", [NHI, 256], f16)

    NB = 4
    assert NT % NB == 0 and B % (NB * P) == 0

    with tile.TileContext(nc) as tc:
        with (
            tc.tile_pool(name="const", bufs=1) as cp,
            tc.tile_pool(name="sbuf", bufs=4) as sb,
            tc.tile_pool(name="gp", bufs=3) as gpool,
            tc.tile_pool(name="big", bufs=2) as bigp,
            tc.tile_pool(name="psA", bufs=2, space="PSUM") as psA,
        ):
            nc.gpsimd.load_library(library_config.mlp)
            wkv_sb = cp.tile([D, 256], f16, tag="wkv")
            wqs_sb = cp.tile([D, 256], f16, tag="wqs")
            q_loc = cp.tile([P, TPC * D], f16, tag="qloc")
            s_loc = cp.tile([P, TPC * D], f16, tag="sloc")
            idx_all = cp.tile([P, WTOT], i16, tag="idxall")
            msk_all = cp.tile([P, KTOT], f16, tag="mskall")
            nc.sync.dma_start(out=wkv_sb[:, 0:128], in_=wkT[:])
            nc.sync.dma_start(out=wkv_sb[:, 128:256], in_=wvT[:])
            nc.sync.dma_start(out=wqs_sb[:, 0:128], in_=wqT[:])
            nc.sync.dma_start(out=wqs_sb[:, 128:256], in_=wsT[:])
            nc.sync.dma_start(out=idx_all[:], in_=idx_w[:])
            nc.sync.dma_start(out=msk_all[:], in_=msk_d[:])

            # ---------------- node phase: kv tables ----------------
            for it in range(NT // NB):
                t0 = it * NB
                xT = sb.tile([P, NB * P], f16, tag="xT")
                nc.sync.dma_start(
                    out=xT[:], in_=x_perm[t0 * P:(t0 + NB) * P, :], transpose=True)
                pkv = psA.tile([P, NB * 256], f32, tag="pbig")
                for b in range(NB):
                    nc.tensor.matmul(pkv[:, b * 256:(b + 1) * 256],
                                     lhsT=xT[:, b * P:(b + 1) * P], rhs=wkv_sb[:],
                                     start=True, stop=True)
                kvt = sb.tile([P, NB * 256], f16, tag="kvt")
                nc.scalar.activation(out=kvt[:], in_=pkv[:], func=COPY)
                r0 = t0 * P
                tgt = (kv_lo[r0:r0 + NB * P, :] if r0 < B
                       else kv_hi[r0 - B:r0 - B + NB * P, :])
                nc.sync.dma_start(
                    out=tgt.rearrange("(b p) c -> p b c", p=P),
                    in_=kvt[:].rearrange("p (b c) -> p b c", c=256))

            # ---------------- local phase: q and skip ----------------
            u = 0
            while u < TPC:
                lb = min(NB, TPC - u)
                xTl = sb.tile([P, NB * P], f16, tag="xT")
                nc.sync.dma_start(
                    out=xTl[:, :lb * P], in_=x_loc[u * P:(u + lb) * P, :],
                    transpose=True)
                pq = psA.tile([P, NB * 256], f32, tag="pbig")
                for b in range(lb):
                    nc.tensor.matmul(pq[:, b * 256:(b + 1) * 256],
                                     lhsT=xTl[:, b * P:(b + 1) * P], rhs=wqs_sb[:],
                                     start=True, stop=True)
                nc.scalar.activation(
                    out=q_loc[:, u * D:(u + lb) * D].rearrange(
                        "p (b c) -> p b c", c=P),
                    in_=pq[:, :lb * 256].rearrange(
                        "p (b c) -> p b c", c=256)[:, :, 0:128], func=COPY)
                xl = sb.tile([P, NB, P], f16, tag="xl")
                nc.sync.dma_start(
                    out=xl[:, :lb, :],
                    in_=x_loc[u * P:(u + lb) * P, :].rearrange(
                        "(b p) c -> p b c", p=P))
                nc.vector.tensor_tensor(
                    out=s_loc[:, u * D:(u + lb) * D].rearrange(
                        "p (b c) -> p b c", c=P),
                    in0=pq[:, :lb * 256].rearrange(
                        "p (b c) -> p b c", c=256)[:, :, 128:256],
                    in1=xl[:, :lb, :], op=ADD)
                u += lb

            # ---------------- edge phase ----------------
            woff = 0
            koff = 0
            gq = 0
            for u in range(TPC):
                KL, KH = Klo[u], Khi[u]
                KT = KL + KH
                if KT == 0:
                    of = sb.tile([P, D], f32, tag="of")
                    nc.scalar.activation(
                        out=of[:], in_=s_loc[:, u * D:(u + 1) * D], func=COPY)
                    nc.sync.dma_start(out=out_t[u * P:(u + 1) * P, :], in_=of[:])
                    continue
                kv_g = gpool.tile([P, Kmax, 256], f16, tag="kvg")
                wo = woff
                for (j0, csz) in _chunks(KL):
                    nc.gpsimd.dma_gather(
                        kv_g[:, j0:j0 + csz, :], kv_lo[:, :],
                        idx_all[:, wo:wo + 8 * csz], csz * P, csz * P, 256,
                        queue_num=gq % 4)
                    gq += 1
                    wo += 8 * csz
                for (j0, csz) in _chunks(KH):
                    nc.gpsimd.dma_gather(
                        kv_g[:, KL + j0:KL + j0 + csz, :], kv_hi[:, :],
                        idx_all[:, wo:wo + 8 * csz], csz * P, csz * P, 256,
                        queue_num=gq % 4)
                    gq += 1
                    wo += 8 * csz
                msk = msk_all[:, koff:koff + KT, None]
                woff += 8 * KT
                koff += KT

                qk = bigp.tile([P, Kmax, D], f16, tag="qk")
                nc.vector.tensor_tensor(
                    out=qk[:, :KT, :], in0=kv_g[:, :KT, 0:D],
                    in1=q_loc[:, u * D:(u + 1) * D][:, None, :].to_broadcast(
                        [P, KT, D]), op=MUL)
                qkh = qk[:, :KT, :].rearrange("p k (h e) -> p (k h) e", e=DH)
                w = DH
                while w > 1:
                    w //= 2
                    nc.vector.tensor_tensor(out=qkh[:, :, 0:w], in0=qkh[:, :, 0:w],
                                            in1=qkh[:, :, w:2 * w], op=ADD)
                alpha = sb.tile([P, Kmax, H], f16, tag="alpha")
                nc.vector.tensor_tensor(
                    out=alpha[:, :KT, :],
                    in0=qkh[:, :, 0:1].rearrange("p (k h) e -> p k (h e)", h=H),
                    in1=msk.to_broadcast([P, KT, H]), op=ADD)
                X = bigp.tile([P, Kmax, 132], f16, tag="X")
                nc.scalar.activation(out=X[:, :KT, 128:132], in_=alpha[:, :KT, :],
                                     func=EXP)
                nc.vector.tensor_tensor(
                    out=X[:, :KT, 0:128].rearrange("p k (h e) -> p k h e", e=DH),
                    in0=kv_g[:, :KT, 128:256].rearrange(
                        "p k (h e) -> p k h e", e=DH),
                    in1=X[:, :KT, 128:132, None].to_broadcast([P, KT, H, DH]),
                    op=MUL)
                # tree-reduce over the KT slots
                w = 1
                while w < KT:
                    w *= 2
                w //= 2
                while w >= 1:
                    lim = min(2 * w, KT)
                    if lim > w:
                        nc.vector.tensor_tensor(
                            out=X[:, 0:lim - w, :], in0=X[:, 0:lim - w, :],
                            in1=X[:, w:lim, :], op=ADD)
                    w //= 2
                rc = sb.tile([P, H], f32, tag="rc")
                nc.vector.tensor_scalar(out=rc[:], in0=X[:, 0, 128:132],
                                        scalar1=1e-16, scalar2=None, op0=ADD)
                nc.vector.reciprocal(out=rc[:], in_=rc[:])
                ot = sb.tile([P, D], f32, tag="ot")
                nc.vector.tensor_tensor(
                    out=ot[:].rearrange("p (h e) -> p h e", e=DH),
                    in0=X[:, 0, 0:128].rearrange("p (h e) -> p h e", e=DH),
                    in1=rc[:, :, None].to_broadcast([P, H, DH]), op=MUL)
                of = sb.tile([P, D], f32, tag="of")
                nc.vector.tensor_tensor(
                    out=of[:], in0=ot[:], in1=s_loc[:, u * D:(u + 1) * D], op=ADD)
                nc.sync.dma_start(out=out_t[u * P:(u + 1) * P, :], in_=of[:])

    nc.finalize()
    return nc


def _run(inputs, trace=False):
    _ensure_hooks()
    from concourse.bass_utils import run_bass_kernel_spmd

    meta = _prep(**inputs)
    key = (meta['TPC'], meta['NT'], meta['Klo'], meta['Khi'],
           meta['WTOT'], meta['KTOT'])
    if key not in _prog_cache:
        _prog_cache[key] = _build(*key)
    nc = _prog_cache[key]
    res = run_bass_kernel_spmd(nc, meta['in_maps'],
                               core_ids=list(range(NCORES)), trace=trace)
    TPC, N = meta['TPC'], meta['N']
    RP = NCORES * P
    out_sorted = np.empty((TPC * RP, D), dtype=np.float32)
    for c in range(NCORES):
        oc = np.asarray(res.results[c]["out"]).reshape(TPC, P, D)
        for u in range(TPC):
            out_sorted[u * RP + c * P:u * RP + (c + 1) * P] = oc[u]
    out = np.empty((N, D), dtype=np.float32)
    out[meta['dkey']] = out_sorted[:N]
    return out, res


def kernel(**inputs) -> np.ndarray:
    out, _ = _run(inputs, trace=False)
    return out


# revision 18
# speedup vs baseline: 1.5414x; 1.1601x over previous
"""TransformerConv MixerBlock (x + TransformerConv(x, edge_index)) on 8 trn2 NeuronCores.

Strategy: kv table rows in degree-sorted order, split at 32768 into lo/hi DRAM
tables so dma_gather's int16 indices reach every row. Destination nodes are
packed into 128-node tiles by (deg_lo, deg_hi) lexsort (rounds of 8 tiles share
slot counts so all cores run one SPMD program). Each core builds the full K/V
table (fp16, 512B rows) from x, then per local tile: a few dma_gather calls
fetch the [128, Klo+Khi] incident kv rows (per-node slots, dummy idx 0 +
mask), alpha/softmax/weighted-sum run as pure vector ops aligned per partition
(no one-hot matmuls), then normalize + skip + residual.
"""
import sys, os, types, math
sys.path.insert(0, '/opt/trn_rl_repo')
import numpy as np

P = 128
D = 128
H = 4
DH = 32
NCORES = 8
B = 32768          # lo/hi table split
SMAX = 8           # max slots (128 idx each) per dma_gather call

_prog_cache = {}


def _ensure_hooks():
    """Best-effort shim of antenv.axon_hooks so trace=True profiling works."""
    try:
        import antenv
        if 'antenv.axon_hooks' not in sys.modules:
            mod = types.ModuleType('antenv.axon_hooks')
            state = {'hook': None}
            mod.set_axon_ntff_profile_hook = lambda h: state.__setitem__('hook', h)
            mod.get_axon_ntff_profile_hook = lambda: state['hook']
            sys.modules['antenv.axon_hooks'] = mod
            antenv.axon_hooks = mod
            from trn_agent_boot.trn_boot import _ntff_profile_via_ctypes
            hook = _ntff_profile_via_ctypes('/opt/axon/libaxon_pjrt.so')
            if hook is not None:
                mod.set_axon_ntff_profile_hook(hook)
    except Exception:
        pass
    try:
        import concourse.bass_utils as bass_utils
        bass_utils.upload_artifacts = lambda tmpdir: tmpdir
    except Exception:
        pass


def _chunks(n):
    """Split n slots into gather chunks of <= SMAX slots."""
    out = []
    j = 0
    while j < n:
        c = min(SMAX, n - j)
        out.append((j, c))
        j += c
    return out


def _wrap16(vals):
    """[128*c] gather list -> wrapped [128, 8*c] int16 (pos i at [i%16, i//16],
    replicated across the 8 groups of 16 partitions)."""
    w = vals.reshape(-1, 16).T.astype(np.int16)      # [16, 8c]
    return np.tile(w, (8, 1))


def _prep(x, edge_index, Wq, bq, Wk, bk, Wv, bv, Wskip, bskip):
    N = x.shape[0]
    E = edge_index.shape[1]
    TPC = (N + NCORES * P - 1) // (NCORES * P)
    NT = NCORES * TPC

    src = np.asarray(edge_index[0], dtype=np.int64)
    dst = np.asarray(edge_index[1], dtype=np.int64)
    deg = np.bincount(dst, minlength=N)

    # --- table rows: degree-sorted; node n -> row trow[n]
    order = np.argsort(-deg, kind='stable')
    trow = np.empty(N, dtype=np.int64)
    trow[order] = np.arange(N)
    x_perm = np.zeros((NT * P, D), dtype=np.float16)
    x_perm[trow] = np.asarray(x, dtype=np.float16)

    # --- per-node lo/hi degrees under the table split
    src_row = trow[src]
    is_lo = src_row < B
    dlo = np.bincount(dst[is_lo], minlength=N)
    dhi = np.bincount(dst[~is_lo], minlength=N)

    # --- dst tiling: lexsort by (dlo desc, dhi desc); position i -> node dkey[i]
    dkey = np.lexsort((-dhi, -dlo))
    dpos = np.empty(N, dtype=np.int64)
    dpos[dkey] = np.arange(N)
    dlo_s = np.zeros(NT * P, dtype=np.int64)
    dhi_s = np.zeros(NT * P, dtype=np.int64)
    dlo_s[:N] = dlo[dkey]
    dhi_s[:N] = dhi[dkey]

    RP = NCORES * P   # 1024 positions per round
    Klo = [int(dlo_s[u * RP:(u + 1) * RP].max()) for u in range(TPC)]
    Khi = [int(dhi_s[u * RP:(u + 1) * RP].max()) for u in range(TPC)]

    # --- per-edge slot assignment ---------------------------------------
    # rank of each edge within its (dst, lo/hi) group
    ord2 = np.lexsort((src_row, np.where(is_lo, 0, 1), dpos[dst]))
    dpos_s = dpos[dst][ord2]
    islo_s = is_lo[ord2]
    srow_s = src_row[ord2]
    grp = dpos_s * 2 + (1 - islo_s)           # lo group first within node
    changes = np.ones(E, dtype=bool)
    changes[1:] = grp[1:] != grp[:-1]
    gstart = np.where(changes)[0]
    gid = np.cumsum(changes) - 1
    rank = np.arange(E) - gstart[gid]

    # idx value: lo -> row, hi -> row - B;   slot j: lo j=rank, hi j=Klo+rank
    u_e = dpos_s // RP
    Klo_e = np.asarray(Klo, dtype=np.int64)[u_e]
    slot = np.where(islo_s, rank, Klo_e + rank)
    ival = np.where(islo_s, srow_s, srow_s - B).astype(np.int16)

    Kmax = max(Klo[u] + Khi[u] for u in range(TPC))
    idx_big = np.zeros((NT * P, Kmax), dtype=np.int16)
    idx_big[dpos_s, slot] = ival
    jr = np.arange(Kmax)[None, :]
    # real slots get -2.0 (uniform logit shift, cancels in the softmax ratio,
    # guards the fp16 denominator sum against overflow); padded slots -10000.
    msk_big = np.where(
        (jr < dlo_s[:, None]) |
        ((jr >= np.repeat(Klo, RP)[:, None]) &
         (jr < (np.repeat(Klo, RP)[:, None] + dhi_s[:, None]))),
        np.float16(-2.0), np.float16(-10000.0))
    idx_big = idx_big.reshape(NT, P, Kmax)
    msk_big = msk_big.reshape(NT, P, Kmax)

    s = 1.0 / math.sqrt(DH)
    wkT = np.asarray(Wk, dtype=np.float32).T.astype(np.float16).copy()
    wvT = np.asarray(Wv, dtype=np.float32).T.astype(np.float16).copy()
    wqT = (np.asarray(Wq, dtype=np.float32).T * s).astype(np.float16).copy()
    wsT = np.asarray(Wskip, dtype=np.float32).T.astype(np.float16).copy()
    for b in (bq, bk, bv, bskip):
        assert np.abs(np.asarray(b)).max() == 0.0, "nonzero biases not supported"

    # wrapped idx stream width per round
    WU = [8 * (sum(c for _, c in _chunks(Klo[u])) +
               sum(c for _, c in _chunks(Khi[u]))) for u in range(TPC)]
    WTOT = sum(WU)
    KTOT = sum(Klo[u] + Khi[u] for u in range(TPC))

    in_maps = []
    for c in range(NCORES):
        # global tile of (core c, local u) covers sorted positions
        # [u*RP + c*P, u*RP + (c+1)*P)
        tidx = [u * NCORES + 0 for u in range(TPC)]  # placeholder
        idx_w = np.empty((P, WTOT), dtype=np.int16)
        msk_c = np.empty((P, KTOT), dtype=np.float16)
        woff = 0
        koff = 0
        xl_rows = np.empty((TPC * P, D), dtype=np.float16)
        for u in range(TPC):
            p0 = u * RP + c * P
            tile_idx = idx_big.reshape(NT * P, Kmax)[p0:p0 + P]
            tile_msk = msk_big.reshape(NT * P, Kmax)[p0:p0 + P]
            for (j0, csz) in _chunks(Klo[u]):
                vals = tile_idx[:, j0:j0 + csz].T.reshape(-1)  # slot-major
                idx_w[:, woff:woff + 8 * csz] = _wrap16(vals)
                woff += 8 * csz
            for (j0, csz) in _chunks(Khi[u]):
                vals = tile_idx[:, Klo[u] + j0:Klo[u] + j0 + csz].T.reshape(-1)
                idx_w[:, woff:woff + 8 * csz] = _wrap16(vals)
                woff += 8 * csz
            kt = Klo[u] + Khi[u]
            msk_c[:, koff:koff + kt] = tile_msk[:, :kt]
            koff += kt
            rows = dkey[p0:p0 + P] if p0 + P <= N else None
            if rows is None:
                seg = dkey[p0:min(p0 + P, N)]
                blk = np.zeros((P, D), dtype=np.float16)
                blk[:len(seg)] = np.asarray(x, dtype=np.float16)[seg]
                xl_rows[u * P:(u + 1) * P] = blk
            else:
                xl_rows[u * P:(u + 1) * P] = np.asarray(x, dtype=np.float16)[rows]
        assert woff == WTOT and koff == KTOT
        in_maps.append({
            "x_perm": x_perm,
            "x_loc": xl_rows,
            "wkT": wkT, "wvT": wvT, "wqT": wqT, "wsT": wsT,
            "idx_w": idx_w, "msk": msk_c,
        })
    return dict(N=N, E=E, TPC=TPC, NT=NT, Klo=tuple(Klo), Khi=tuple(Khi),
                dkey=dkey, WTOT=WTOT, KTOT=KTOT, in_maps=in_maps)


def _build(TPC, NT, Klo, Khi, WTOT, KTOT):
    import concourse.bass as bass
    import concourse.bacc as bacc
    import concourse.mybir as mybir
    import concourse.tile as tile
    from concourse import library_config

    f16 = mybir.dt.float16
    f32 = mybir.dt.float32
    i16 = mybir.dt.int16
    MUL = mybir.AluOpType.mult
    ADD = mybir.AluOpType.add
    EXP = mybir.ActivationFunctionType.Exp
    COPY = mybir.ActivationFunctionType.Copy
    AXLX = mybir.AxisListType.X

    Kmax = max(Klo[u] + Khi[u] for u in range(TPC))
    NHI = NT * P - B

    nc = bacc.Bacc("TRN2", target_bir_lowering=False, debug=False,
                   num_swdge_queues=4)
    x_perm = nc.dram_tensor("x_perm", [NT * P, D], f16, kind="ExternalInput")
    x_loc = nc.dram_tensor("x_loc", [TPC * P, D], f16, kind="ExternalInput")
    wkT = nc.dram_tensor("wkT", [D, D], f16, kind="ExternalInput")
    wvT = nc.dram_tensor("wvT", [D, D], f16, kind="ExternalInput")
    wqT = nc.dram_tensor("wqT", [D, D], f16, kind="ExternalInput")
    wsT = nc.dram_tensor("wsT", [D, D], f16, kind="ExternalInput")
    idx_w = nc.dram_tensor("idx_w", [P, WTOT], i16, kind="ExternalInput")
    msk_d = nc.dram_tensor("msk", [P, KTOT], f16, kind="ExternalInput")
    out_t = nc.dram_tensor("out", [TPC * P, D], f32, kind="ExternalOutput")

    kv_lo = nc.dram_tensor("kv_lo", [B, 256], f16)
    kv_hi = nc.dram_tensor("kv_hi", [NHI, 256], f16)

    NB = 4
    assert NT % NB == 0 and B % (NB * P) == 0

    with tile.TileContext(nc) as tc:
        with (
            tc.tile_pool(name="const", bufs=1) as cp,
            tc.tile_pool(name="sbuf", bufs=4) as sb,
            tc.tile_pool(name="gp", bufs=3) as gpool,
            tc.tile_pool(name="big", bufs=2) as bigp,
            tc.tile_pool(name="psA", bufs=2, space="PSUM") as psA,
        ):
            nc.gpsimd.load_library(library_config.mlp)
            wkv_sb = cp.tile([D, 256], f16, tag="wkv")
            wqs_sb = cp.tile([D, 256], f16, tag="wqs")
            q_loc = cp.tile([P, TPC * D], f16, tag="qloc")
            s_loc = cp.tile([P, TPC * D], f16, tag="sloc")
            idx_all = cp.tile([P, WTOT], i16, tag="idxall")
            msk_all = cp.tile([P, KTOT], f16, tag="mskall")
            nc.sync.dma_start(out=wkv_sb[:, 0:128], in_=wkT[:])
            nc.sync.dma_start(out=wkv_sb[:, 128:256], in_=wvT[:])
            nc.sync.dma_start(out=wqs_sb[:, 0:128], in_=wqT[:])
            nc.sync.dma_start(out=wqs_sb[:, 128:256], in_=wsT[:])
            nc.sync.dma_start(out=idx_all[:], in_=idx_w[:])
            nc.sync.dma_start(out=msk_all[:], in_=msk_d[:])

            # ---------------- local phase: q and skip ----------------
            u = 0
            while u < TPC:
                lb = min(NB, TPC - u)
                xTl = sb.tile([P, NB * P], f16, tag="xT")
                nc.sync.dma_start(
                    out=xTl[:, :lb * P], in_=x_loc[u * P:(u + lb) * P, :],
                    transpose=True)
                pq = psA.tile([P, NB * 256], f32, tag="pbig")
                for b in range(lb):
                    nc.tensor.matmul(pq[:, b * 256:(b + 1) * 256],
                                     lhsT=xTl[:, b * P:(b + 1) * P], rhs=wqs_sb[:],
                                     start=True, stop=True)
                nc.scalar.activation(
                    out=q_loc[:, u * D:(u + lb) * D].rearrange(
                        "p (b c) -> p b c", c=P),
                    in_=pq[:, :lb * 256].rearrange(
                        "p (b c) -> p b c", c=256)[:, :, 0:128], func=COPY)
                xl = sb.tile([P, NB, P], f16, tag="xl")
                nc.sync.dma_start(
                    out=xl[:, :lb, :],
                    in_=x_loc[u * P:(u + lb) * P, :].rearrange(
                        "(b p) c -> p b c", p=P))
                nc.vector.tensor_tensor(
                    out=s_loc[:, u * D:(u + lb) * D].rearrange(
                        "p (b c) -> p b c", c=P),
                    in0=pq[:, :lb * 256].rearrange(
                        "p (b c) -> p b c", c=256)[:, :, 128:256],
                    in1=xl[:, :lb, :], op=ADD)
                u += lb

            # ---------------- node phase: kv tables ----------------
            NB2 = 16
            assert NT % NB == 0
            for tt0 in range(0, NT, NB2):
                lb2 = min(NB2, NT - tt0)
                xT = sb.tile([P, NB2 * P], f16, tag="xT2")
                nc.sync.dma_start(
                    out=xT[:, :lb2 * P], in_=x_perm[tt0 * P:(tt0 + lb2) * P, :],
                    transpose=True)
                for g in range(lb2 // NB):
                    t0 = tt0 + g * NB
                    pkv = psA.tile([P, NB * 256], f32, tag="pbig")
                    for b in range(NB):
                        nc.tensor.matmul(
                            pkv[:, b * 256:(b + 1) * 256],
                            lhsT=xT[:, (g * NB + b) * P:(g * NB + b + 1) * P],
                            rhs=wkv_sb[:], start=True, stop=True)
                    kvt = sb.tile([P, NB * 256], f16, tag="kvt")
                    if g % 2 == 0:
                        nc.scalar.activation(out=kvt[:], in_=pkv[:], func=COPY)
                    else:
                        nc.vector.tensor_scalar(out=kvt[:], in0=pkv[:],
                                                scalar1=0.0, scalar2=None,
                                                op0=ADD)
                    r0 = t0 * P
                    tgt = (kv_lo[r0:r0 + NB * P, :] if r0 < B
                           else kv_hi[r0 - B:r0 - B + NB * P, :])
                    nc.sync.dma_start(
                        out=tgt.rearrange("(b p) c -> p b c", p=P),
                        in_=kvt[:].rearrange("p (b c) -> p b c", c=256))

            # ---------------- edge phase ----------------
            woff = 0
            koff = 0
            gq = 0
            for u in range(TPC):
                KL, KH = Klo[u], Khi[u]
                KT = KL + KH
                if KT == 0:
                    of = sb.tile([P, D], f32, tag="of")
                    nc.scalar.activation(
                        out=of[:], in_=s_loc[:, u * D:(u + 1) * D], func=COPY)
                    nc.sync.dma_start(out=out_t[u * P:(u + 1) * P, :], in_=of[:])
                    continue
                kv_g = gpool.tile([P, Kmax, 256], f16, tag="kvg")
                wo = woff
                for (j0, csz) in _chunks(KL):
                    nc.gpsimd.dma_gather(
                        kv_g[:, j0:j0 + csz, :], kv_lo[:, :],
                        idx_all[:, wo:wo + 8 * csz], csz * P, csz * P, 256,
                        queue_num=gq % 4)
                    gq += 1
                    wo += 8 * csz
                for (j0, csz) in _chunks(KH):
                    nc.gpsimd.dma_gather(
                        kv_g[:, KL + j0:KL + j0 + csz, :], kv_hi[:, :],
                        idx_all[:, wo:wo + 8 * csz], csz * P, csz * P, 256,
                        queue_num=gq % 4)
                    gq += 1
                    wo += 8 * csz
                msk = msk_all[:, koff:koff + KT, None]
                woff += 8 * KT
                koff += KT

                qk = bigp.tile([P, Kmax, D], f16, tag="qk")
                nc.vector.tensor_tensor(
                    out=qk[:, :KT, :], in0=kv_g[:, :KT, 0:D],
                    in1=q_loc[:, u * D:(u + 1) * D][:, None, :].to_broadcast(
                        [P, KT, D]), op=MUL)
                qkh = qk[:, :KT, :].rearrange("p k (h e) -> p (k h) e", e=DH)
                w = DH
                while w > 1:
                    w //= 2
                    nc.vector.tensor_tensor(out=qkh[:, :, 0:w], in0=qkh[:, :, 0:w],
                                            in1=qkh[:, :, w:2 * w], op=ADD)
                alpha = sb.tile([P, Kmax, H], f16, tag="alpha")
                nc.vector.tensor_tensor(
                    out=alpha[:, :KT, :],
                    in0=qkh[:, :, 0:1].rearrange("p (k h) e -> p k (h e)", h=H),
                    in1=msk.to_broadcast([P, KT, H]), op=ADD)
                a_e = sb.tile([P, Kmax, H], f16, tag="a_e")
                nc.scalar.activation(out=a_e[:, :KT, :], in_=alpha[:, :KT, :],
                                     func=EXP)
                X = bigp.tile([P, Kmax, D], f16, tag="X")
                nc.vector.tensor_tensor(
                    out=X[:, :KT, :].rearrange("p k (h e) -> p k h e", e=DH),
                    in0=kv_g[:, :KT, 128:256].rearrange(
                        "p k (h e) -> p k h e", e=DH),
                    in1=a_e[:, :KT, :, None].to_broadcast([P, KT, H, DH]),
                    op=MUL)
                # tree-reduce over the KT slots (values and denominators)
                w = 1
                while w < KT:
                    w *= 2
                w //= 2
                while w >= 1:
                    lim = min(2 * w, KT)
                    if lim > w:
                        nc.vector.tensor_tensor(
                            out=X[:, 0:lim - w, :], in0=X[:, 0:lim - w, :],
                            in1=X[:, w:lim, :], op=ADD)
                        nc.vector.tensor_tensor(
                            out=a_e[:, 0:lim - w, :], in0=a_e[:, 0:lim - w, :],
                            in1=a_e[:, w:lim, :], op=ADD)
                    w //= 2
                rc = sb.tile([P, H], f32, tag="rc")
                nc.vector.tensor_scalar(out=rc[:], in0=a_e[:, 0, :],
                                        scalar1=1e-16, scalar2=None, op0=ADD)
                nc.vector.reciprocal(out=rc[:], in_=rc[:])
                ot = sb.tile([P, D], f32, tag="ot")
                nc.vector.tensor_tensor(
                    out=ot[:].rearrange("p (h e) -> p h e", e=DH),
                    in0=X[:, 0, :].rearrange("p (h e) -> p h e", e=DH),
                    in1=rc[:, :, None].to_broadcast([P, H, DH]), op=MUL)
                of = sb.tile([P, D], f32, tag="of")
                nc.vector.tensor_tensor(
                    out=of[:], in0=ot[:], in1=s_loc[:, u * D:(u + 1) * D], op=ADD)
                nc.sync.dma_start(out=out_t[u * P:(u + 1) * P, :], in_=of[:])

    nc.finalize()
    return nc


def _run(inputs, trace=False):
    _ensure_hooks()
    from concourse.bass_utils import run_bass_kernel_spmd

    meta = _prep(**inputs)
    key = (meta['TPC'], meta['NT'], meta['Klo'], meta['Khi'],
           meta['WTOT'], meta['KTOT'])
    if key not in _prog_cache:
        _prog_cache[key] = _build(*key)
    nc = _prog_cache[key]
    res = run_bass_kernel_spmd(nc, meta['in_maps'],
                               core_ids=list(range(NCORES)), trace=trace)
    TPC, N = meta['TPC'], meta['N']
    RP = NCORES * P
    out_sorted = np.empty((TPC * RP, D), dtype=np.float32)
    for c in range(NCORES):
        oc = np.asarray(res.results[c]["out"]).reshape(TPC, P, D)
        for u in range(TPC):
            out_sorted[u * RP + c * P:u * RP + (c + 1) * P] = oc[u]
    out = np.empty((N, D), dtype=np.float32)
    out[meta['dkey']] = out_sorted[:N]
    return out, res


def kernel(**inputs) -> np.ndarray:
    out, _ = _run(inputs, trace=False)
    return out


# revision 30
# speedup vs baseline: 1.9135x; 1.2414x over previous
"""TransformerConv MixerBlock (x + TransformerConv(x, edge_index)) on 8 trn2 NeuronCores.

Strategy: kv table rows in degree-sorted order, split at 32768 into lo/hi DRAM
tables so dma_gather's int16 indices reach every row. Destination nodes are
packed into 128-node tiles by (deg_lo, deg_hi) lexsort (rounds of 8 tiles share
slot counts so all cores run one SPMD program). Each core builds the full K/V
table (fp16, 512B rows) from x, then per local tile: a few dma_gather calls
fetch the [128, Klo+Khi] incident kv rows (per-node slots, dummy idx 0 +
mask), alpha/softmax/weighted-sum run as pure vector ops aligned per partition
(no one-hot matmuls), then normalize + skip + residual.
"""
import sys, os, types, math
sys.path.insert(0, '/opt/trn_rl_repo')
import numpy as np

P = 128
D = 128
H = 4
DH = 32
NCORES = 8
B = 32768          # lo/hi table split
SMAX = 8           # max slots (128 idx each) per dma_gather call

_prog_cache = {}


def _ensure_hooks():
    """Best-effort shim of antenv.axon_hooks so trace=True profiling works."""
    try:
        import antenv
        if 'antenv.axon_hooks' not in sys.modules:
            mod = types.ModuleType('antenv.axon_hooks')
            state = {'hook': None}
            mod.set_axon_ntff_profile_hook = lambda h: state.__setitem__('hook', h)
            mod.get_axon_ntff_profile_hook = lambda: state['hook']
            sys.modules['antenv.axon_hooks'] = mod
            antenv.axon_hooks = mod
            from trn_agent_boot.trn_boot import _ntff_profile_via_ctypes
            hook = _ntff_profile_via_ctypes('/opt/axon/libaxon_pjrt.so')
            if hook is not None:
                mod.set_axon_ntff_profile_hook(hook)
    except Exception:
        pass
    try:
        import concourse.bass_utils as bass_utils
        bass_utils.upload_artifacts = lambda tmpdir: tmpdir
    except Exception:
        pass


def _chunks(n):
    """Split n slots into gather chunks of <= SMAX slots."""
    out = []
    j = 0
    while j < n:
        c = min(SMAX, n - j)
        out.append((j, c))
        j += c
    return out


def _wrap16(vals):
    """[128*c] gather list -> wrapped [128, 8*c] int16 (pos i at [i%16, i//16],
    replicated across the 8 groups of 16 partitions)."""
    w = vals.reshape(-1, 16).T.astype(np.int16)      # [16, 8c]
    return np.tile(w, (8, 1))


def _prep(x, edge_index, Wq, bq, Wk, bk, Wv, bv, Wskip, bskip):
    N = x.shape[0]
    E = edge_index.shape[1]
    TPC = (N + NCORES * P - 1) // (NCORES * P)
    NT = NCORES * TPC

    src = np.asarray(edge_index[0], dtype=np.int64)
    dst = np.asarray(edge_index[1], dtype=np.int64)
    deg = np.bincount(dst, minlength=N)

    # --- table rows: degree-sorted; node n -> row trow[n]
    order = np.argsort(-deg, kind='stable')
    trow = np.empty(N, dtype=np.int64)
    trow[order] = np.arange(N)
    x_perm = np.zeros((NT * P, D), dtype=np.float16)
    x_perm[trow] = np.asarray(x, dtype=np.float16)

    # --- per-node lo/hi degrees under the table split
    src_row = trow[src]
    is_lo = src_row < B
    dlo = np.bincount(dst[is_lo], minlength=N)
    dhi = np.bincount(dst[~is_lo], minlength=N)

    # --- dst tiling: lexsort by (dlo desc, dhi desc); position i -> node dkey[i]
    dkey = np.lexsort((-dhi, -dlo))
    dpos = np.empty(N, dtype=np.int64)
    dpos[dkey] = np.arange(N)
    dlo_s = np.zeros(NT * P, dtype=np.int64)
    dhi_s = np.zeros(NT * P, dtype=np.int64)
    dlo_s[:N] = dlo[dkey]
    dhi_s[:N] = dhi[dkey]

    RP = NCORES * P   # 1024 positions per round
    Klo = [int(dlo_s[u * RP:(u + 1) * RP].max()) for u in range(TPC)]
    Khi = [int(dhi_s[u * RP:(u + 1) * RP].max()) for u in range(TPC)]

    # --- per-edge slot assignment ---------------------------------------
    # rank of each edge within its (dst, lo/hi) group
    ord2 = np.lexsort((src_row, np.where(is_lo, 0, 1), dpos[dst]))
    dpos_s = dpos[dst][ord2]
    islo_s = is_lo[ord2]
    srow_s = src_row[ord2]
    grp = dpos_s * 2 + (1 - islo_s)           # lo group first within node
    changes = np.ones(E, dtype=bool)
    changes[1:] = grp[1:] != grp[:-1]
    gstart = np.where(changes)[0]
    gid = np.cumsum(changes) - 1
    rank = np.arange(E) - gstart[gid]

    # idx value: lo -> row, hi -> row - B;   slot j: lo j=rank, hi j=Klo+rank
    u_e = dpos_s // RP
    Klo_e = np.asarray(Klo, dtype=np.int64)[u_e]
    slot = np.where(islo_s, rank, Klo_e + rank)
    ival = np.where(islo_s, srow_s, srow_s - B).astype(np.int16)

    Kmax = max(Klo[u] + Khi[u] for u in range(TPC))
    idx_big = np.zeros((NT * P, Kmax), dtype=np.int16)
    idx_big[dpos_s, slot] = ival
    jr = np.arange(Kmax)[None, :]
    # real slots get -2.0 (uniform logit shift, cancels in the softmax ratio,
    # guards the fp16 denominator sum against overflow); padded slots -10000.
    msk_big = np.where(
        (jr < dlo_s[:, None]) |
        ((jr >= np.repeat(Klo, RP)[:, None]) &
         (jr < (np.repeat(Klo, RP)[:, None] + dhi_s[:, None]))),
        np.float16(-2.0), np.float16(-10000.0))
    idx_big = idx_big.reshape(NT, P, Kmax)
    msk_big = msk_big.reshape(NT, P, Kmax)

    s = 1.0 / math.sqrt(DH)
    wkT = np.asarray(Wk, dtype=np.float32).T.astype(np.float16).copy()
    wvT = np.asarray(Wv, dtype=np.float32).T.astype(np.float16).copy()
    wqT = (np.asarray(Wq, dtype=np.float32).T * s).astype(np.float16).copy()
    wsT = np.asarray(Wskip, dtype=np.float32).T.astype(np.float16).copy()
    for b in (bq, bk, bv, bskip):
        assert np.abs(np.asarray(b)).max() == 0.0, "nonzero biases not supported"

    # wrapped idx stream width per round
    WU = [8 * (sum(c for _, c in _chunks(Klo[u])) +
               sum(c for _, c in _chunks(Khi[u]))) for u in range(TPC)]
    WTOT = sum(WU)
    KTOT = sum(Klo[u] + Khi[u] for u in range(TPC))

    NCH = sum(len(_chunks(Klo[u])) + len(_chunks(Khi[u])) for u in range(TPC))
    xf16 = np.asarray(x, dtype=np.float16)
    in_maps = []
    for c in range(NCORES):
        # global tile of (core c, local u) covers sorted positions
        # [u*RP + c*P, u*RP + (c+1)*P)
        idx_w = np.empty((P, WTOT), dtype=np.int16)
        msk_c = np.empty((P, KTOT), dtype=np.float16)
        cnt_c = np.zeros(NCH, dtype=np.int32)
        woff = 0
        koff = 0
        ci = 0
        xl_rows = np.empty((TPC * P, D), dtype=np.float16)
        for u in range(TPC):
            p0 = u * RP + c * P
            tile_idx = idx_big.reshape(NT * P, Kmax)[p0:p0 + P]
            tile_msk = msk_big.reshape(NT * P, Kmax)[p0:p0 + P]
            t_dlo = dlo_s[p0:p0 + P]          # descending within tile
            t_dhi = dhi_s[p0:p0 + P]
            for (j0, csz) in _chunks(Klo[u]):
                vals = tile_idx[:, j0:j0 + csz].T.copy()   # [csz, P] slot-major
                # trailing trim: nodes sorted by dlo desc -> column j's valid
                # entries are a prefix of length count(dlo > j)
                jl = -1
                vcnt = 0
                for j in range(j0, j0 + csz):
                    nj = int((t_dlo > j).sum())
                    if nj > 0:
                        jl = j
                        vcnt = (j - j0) * P + nj
                flat = vals.reshape(-1)
                if vcnt == 0:
                    vcnt = 1          # keep one (masked) dummy: ucode+sim need
                flat[vcnt:] = -1      # a non-empty valid prefix
                cnt_c[ci] = vcnt
                ci += 1
                idx_w[:, woff:woff + 8 * csz] = _wrap16(flat)
                woff += 8 * csz
            for (j0, csz) in _chunks(Khi[u]):
                vals = tile_idx[:, Klo[u] + j0:Klo[u] + j0 + csz].T.copy()
                # dhi not sorted within tile: trim only full trailing columns
                kt_t = int(t_dhi.max())
                jv = max(0, min(csz, kt_t - j0))
                flat = vals.reshape(-1)
                vcnt = jv * P
                if vcnt == 0:
                    vcnt = 1
                flat[vcnt:] = -1
                cnt_c[ci] = vcnt
                ci += 1
                idx_w[:, woff:woff + 8 * csz] = _wrap16(flat)
                woff += 8 * csz
            kt = Klo[u] + Khi[u]
            msk_c[:, koff:koff + kt] = tile_msk[:, :kt]
            koff += kt
            seg = dkey[p0:min(p0 + P, N)]
            blk = np.zeros((P, D), dtype=np.float16)
            blk[:len(seg)] = xf16[seg]
            xl_rows[u * P:(u + 1) * P] = blk
        assert woff == WTOT and koff == KTOT and ci == NCH
        in_maps.append({
            "x_perm": x_perm,
            "x_loc": xl_rows,
            "wkT": wkT, "wvT": wvT, "wqT": wqT, "wsT": wsT,
            "idx_w": idx_w, "msk": msk_c, "cnt": cnt_c.reshape(1, NCH),
        })
    return dict(N=N, E=E, TPC=TPC, NT=NT, Klo=tuple(Klo), Khi=tuple(Khi),
                dkey=dkey, WTOT=WTOT, KTOT=KTOT, NCH=NCH, in_maps=in_maps)


def _build(TPC, NT, Klo, Khi, WTOT, KTOT, NCH):
    import concourse.bass as bass
    import concourse.bacc as bacc
    import concourse.mybir as mybir
    import concourse.tile as tile
    from concourse import library_config

    f16 = mybir.dt.float16
    f32 = mybir.dt.float32
    i16 = mybir.dt.int16
    MUL = mybir.AluOpType.mult
    ADD = mybir.AluOpType.add
    EXP = mybir.ActivationFunctionType.Exp
    COPY = mybir.ActivationFunctionType.Copy
    i32 = mybir.dt.int32

    Kmax = max(Klo[u] + Khi[u] for u in range(TPC))
    NHI = NT * P - B

    nc = bacc.Bacc("TRN2", target_bir_lowering=False, debug=False,
                   num_swdge_queues=4)
    x_perm = nc.dram_tensor("x_perm", [NT * P, D], f16, kind="ExternalInput")
    x_loc = nc.dram_tensor("x_loc", [TPC * P, D], f16, kind="ExternalInput")
    wkT = nc.dram_tensor("wkT", [D, D], f16, kind="ExternalInput")
    wvT = nc.dram_tensor("wvT", [D, D], f16, kind="ExternalInput")
    wqT = nc.dram_tensor("wqT", [D, D], f16, kind="ExternalInput")
    wsT = nc.dram_tensor("wsT", [D, D], f16, kind="ExternalInput")
    idx_w = nc.dram_tensor("idx_w", [P, WTOT], i16, kind="ExternalInput")
    msk_d = nc.dram_tensor("msk", [P, KTOT], f16, kind="ExternalInput")
    cnt_d = nc.dram_tensor("cnt", [1, NCH], i32, kind="ExternalInput")
    out_t = nc.dram_tensor("out", [TPC * P, D], f32, kind="ExternalOutput")

    kv_lo = nc.dram_tensor("kv_lo", [B, 256], f16)
    kv_hi = nc.dram_tensor("kv_hi", [NHI, 256], f16)

    NB = 4
    assert NT % NB == 0 and B % (NB * P) == 0

    with tile.TileContext(nc) as tc:
        with (
            tc.tile_pool(name="const", bufs=1) as cp,
            tc.tile_pool(name="sbuf", bufs=4) as sb,
            tc.tile_pool(name="gp", bufs=3) as gpool,
            tc.tile_pool(name="big", bufs=2) as bigp,
            tc.tile_pool(name="psA", bufs=2, space="PSUM") as psA,
        ):
            nc.gpsimd.load_library(library_config.mlp)
            wkv_sb = cp.tile([D, 256], f16, tag="wkv")
            wqs_sb = cp.tile([D, 256], f16, tag="wqs")
            q_loc = cp.tile([P, TPC * D], f16, tag="qloc")
            s_loc = cp.tile([P, TPC * D], f16, tag="sloc")
            idx_all = cp.tile([P, WTOT], i16, tag="idxall")
            msk_all = cp.tile([P, KTOT], f16, tag="mskall")
            cnt_sb = cp.tile([1, NCH], i32, tag="cnt")
            nc.sync.dma_start(out=cnt_sb[:], in_=cnt_d[:])
            greg = nc.gpsimd.alloc_register("gcnt")
            nc.sync.dma_start(out=wkv_sb[:, 0:128], in_=wkT[:])
            nc.sync.dma_start(out=wkv_sb[:, 128:256], in_=wvT[:])
            nc.sync.dma_start(out=wqs_sb[:, 0:128], in_=wqT[:])
            nc.sync.dma_start(out=wqs_sb[:, 128:256], in_=wsT[:])
            nc.sync.dma_start(out=idx_all[:], in_=idx_w[:])
            nc.sync.dma_start(out=msk_all[:], in_=msk_d[:])

            # ---------------- local phase: q and skip ----------------
            u = 0
            while u < TPC:
                lb = min(NB, TPC - u)
                xTl = sb.tile([P, NB * P], f16, tag="xT")
                nc.sync.dma_start(
                    out=xTl[:, :lb * P], in_=x_loc[u * P:(u + lb) * P, :],
                    transpose=True)
                pq = psA.tile([P, NB * 256], f32, tag="pbig")
                for b in range(lb):
                    nc.tensor.matmul(pq[:, b * 256:(b + 1) * 256],
                                     lhsT=xTl[:, b * P:(b + 1) * P], rhs=wqs_sb[:],
                                     start=True, stop=True)
                nc.scalar.activation(
                    out=q_loc[:, u * D:(u + lb) * D].rearrange(
                        "p (b c) -> p b c", c=P),
                    in_=pq[:, :lb * 256].rearrange(
                        "p (b c) -> p b c", c=256)[:, :, 0:128], func=COPY)
                xl = sb.tile([P, NB, P], f16, tag="xl")
                nc.sync.dma_start(
                    out=xl[:, :lb, :],
                    in_=x_loc[u * P:(u + lb) * P, :].rearrange(
                        "(b p) c -> p b c", p=P))
                nc.vector.tensor_tensor(
                    out=s_loc[:, u * D:(u + lb) * D].rearrange(
                        "p (b c) -> p b c", c=P),
                    in0=pq[:, :lb * 256].rearrange(
                        "p (b c) -> p b c", c=256)[:, :, 128:256],
                    in1=xl[:, :lb, :], op=ADD)
                u += lb

            # ---------------- node phase: kv tables ----------------
            NB2 = 16
            assert NT % NB == 0
            for tt0 in range(0, NT, NB2):
                lb2 = min(NB2, NT - tt0)
                xT = sb.tile([P, NB2 * P], f16, tag="xT2")
                nc.sync.dma_start(
                    out=xT[:, :lb2 * P], in_=x_perm[tt0 * P:(tt0 + lb2) * P, :],
                    transpose=True)
                for g in range(lb2 // NB):
                    t0 = tt0 + g * NB
                    pkv = psA.tile([P, NB * 256], f32, tag="pbig")
                    for b in range(NB):
                        nc.tensor.matmul(
                            pkv[:, b * 256:(b + 1) * 256],
                            lhsT=xT[:, (g * NB + b) * P:(g * NB + b + 1) * P],
                            rhs=wkv_sb[:], start=True, stop=True)
                    kvt = sb.tile([P, NB * 256], f16, tag="kvt")
                    nc.scalar.activation(out=kvt[:, :NB * 128],
                                         in_=pkv[:, :NB * 128], func=COPY)
                    nc.vector.tensor_scalar(out=kvt[:, NB * 128:],
                                            in0=pkv[:, NB * 128:],
                                            scalar1=0.0, scalar2=None, op0=ADD)
                    r0 = t0 * P
                    tgt = (kv_lo[r0:r0 + NB * P, :] if r0 < B
                           else kv_hi[r0 - B:r0 - B + NB * P, :])
                    nc.sync.dma_start(
                        out=tgt.rearrange("(b p) c -> p b c", p=P),
                        in_=kvt[:].rearrange("p (b c) -> p b c", c=256))

            # ---------------- edge phase ----------------
            woff = 0
            koff = 0
            gq = 0
            nkvg = 0
            for u in range(TPC):
                KL, KH = Klo[u], Khi[u]
                KT = KL + KH
                if KT == 0:
                    of = sb.tile([P, D], f32, tag="of")
                    nc.scalar.activation(
                        out=of[:], in_=s_loc[:, u * D:(u + 1) * D], func=COPY)
                    nc.sync.dma_start(out=out_t[u * P:(u + 1) * P, :], in_=of[:])
                    continue
                kv_g = gpool.tile([P, Kmax, 256], f16, tag="kvg")
                if nkvg < 3:
                    # first use of each rotating buffer: zero it so reg-trimmed
                    # gathers leave finite (masked) values in skipped slots
                    nc.vector.memset(kv_g[:], 0.0)
                    nkvg += 1
                wo = woff
                for (j0, csz) in _chunks(KL):
                    nc.gpsimd.reg_load(greg, cnt_sb[0:1, gq:gq + 1])
                    nc.gpsimd.dma_gather(
                        kv_g[:, j0:j0 + csz, :], kv_lo[:, :],
                        idx_all[:, wo:wo + 8 * csz], csz * P, greg, 256,
                        queue_num=gq % 4)
                    gq += 1
                    wo += 8 * csz
                for (j0, csz) in _chunks(KH):
                    nc.gpsimd.reg_load(greg, cnt_sb[0:1, gq:gq + 1])
                    nc.gpsimd.dma_gather(
                        kv_g[:, KL + j0:KL + j0 + csz, :], kv_hi[:, :],
                        idx_all[:, wo:wo + 8 * csz], csz * P, greg, 256,
                        queue_num=gq % 4)
                    gq += 1
                    wo += 8 * csz
                msk = msk_all[:, koff:koff + KT, None]
                woff += 8 * KT
                koff += KT

                qk = bigp.tile([P, Kmax, D], f16, tag="qk")
                nc.vector.tensor_tensor(
                    out=qk[:, :KT, :], in0=kv_g[:, :KT, 0:D],
                    in1=q_loc[:, u * D:(u + 1) * D][:, None, :].to_broadcast(
                        [P, KT, D]), op=MUL)
                qkh = qk[:, :KT, :].rearrange("p k (h e) -> p (k h) e", e=DH)
                w = DH
                while w > 1:
                    w //= 2
                    nc.vector.tensor_tensor(out=qkh[:, :, 0:w], in0=qkh[:, :, 0:w],
                                            in1=qkh[:, :, w:2 * w], op=ADD)
                alpha = sb.tile([P, Kmax, H], f16, tag="alpha")
                nc.vector.tensor_tensor(
                    out=alpha[:, :KT, :],
                    in0=qkh[:, :, 0:1].rearrange("p (k h) e -> p k (h e)", h=H),
                    in1=msk.to_broadcast([P, KT, H]), op=ADD)
                a_e = sb.tile([P, Kmax, H], f16, tag="a_e")
                nc.scalar.activation(out=a_e[:, :KT, :], in_=alpha[:, :KT, :],
                                     func=EXP)
                X = bigp.tile([P, Kmax, D], f16, tag="X")
                nc.vector.tensor_tensor(
                    out=X[:, :KT, :].rearrange("p k (h e) -> p k h e", e=DH),
                    in0=kv_g[:, :KT, 128:256].rearrange(
                        "p k (h e) -> p k h e", e=DH),
                    in1=a_e[:, :KT, :, None].to_broadcast([P, KT, H, DH]),
                    op=MUL)
                # tree-reduce over the KT slots (values and denominators)
                w = 1
                while w < KT:
                    w *= 2
                w //= 2
                while w >= 1:
                    lim = min(2 * w, KT)
                    if lim > w:
                        nc.vector.tensor_tensor(
                            out=X[:, 0:lim - w, :], in0=X[:, 0:lim - w, :],
                            in1=X[:, w:lim, :], op=ADD)
                        nc.vector.tensor_tensor(
                            out=a_e[:, 0:lim - w, :], in0=a_e[:, 0:lim - w, :],
                            in1=a_e[:, w:lim, :], op=ADD)
                    w //= 2
                rc = sb.tile([P, H], f32, tag="rc")
                nc.vector.tensor_scalar(out=rc[:], in0=a_e[:, 0, :],
                                        scalar1=1e-16, scalar2=None, op0=ADD)
                nc.vector.reciprocal(out=rc[:], in_=rc[:])
                ot = sb.tile([P, D], f32, tag="ot")
                nc.vector.tensor_tensor(
                    out=ot[:].rearrange("p (h e) -> p h e", e=DH),
                    in0=X[:, 0, :].rearrange("p (h e) -> p h e", e=DH),
                    in1=rc[:, :, None].to_broadcast([P, H, DH]), op=MUL)
                of = sb.tile([P, D], f32, tag="of")
                nc.vector.tensor_tensor(
                    out=of[:], in0=ot[:], in1=s_loc[:, u * D:(u + 1) * D], op=ADD)
                nc.sync.dma_start(out=out_t[u * P:(u + 1) * P, :], in_=of[:])

    nc.finalize()
    return nc


def _run(inputs, trace=False):
    _ensure_hooks()
    from concourse.bass_utils import run_bass_kernel_spmd

    meta = _prep(**inputs)
    key = (meta['TPC'], meta['NT'], meta['Klo'], meta['Khi'],
           meta['WTOT'], meta['KTOT'], meta['NCH'])
    if key not in _prog_cache:
        _prog_cache[key] = _build(*key)
    nc = _prog_cache[key]
    res = run_bass_kernel_spmd(nc, meta['in_maps'],
                               core_ids=list(range(NCORES)), trace=trace)
    TPC, N = meta['TPC'], meta['N']
    RP = NCORES * P
    out_sorted = np.empty((TPC * RP, D), dtype=np.float32)
    for c in range(NCORES):
        oc = np.asarray(res.results[c]["out"]).reshape(TPC, P, D)
        for u in range(TPC):
            out_sorted[u * RP + c * P:u * RP + (c + 1) * P] = oc[u]
    out = np.empty((N, D), dtype=np.float32)
    out[meta['dkey']] = out_sorted[:N]
    return out, res


def kernel(**inputs) -> np.ndarray:
    out, _ = _run(inputs, trace=False)
    return out
